# revision 1
# baseline (speedup 1.0000x reference)
"""EMD loss kernel for Trainium2 (8 NeuronCores, pure data parallel).

Computes out[b] = sum_t (cumsum(x-y, axis=1)[b, t])^2 for x, y [131072, 256] f32.

Per-core plan (16384 rows each, no cross-core communication):
  - x and y are packed host-side into one [2, 16384, 256] DRAM parameter so
    each streaming chunk is ONE DMA instruction on the qSP HWDGE ring (the
    ring executes its DMAs serially — fewer/bigger instructions = more ring
    throughput — and x/y arrive together).
  - View the shard as [128 partitions, 128 row-blocks, 256 bins]; per
    row-block a VectorE tensor_tensor_scan computes the running CDF
    difference state = (x_t + state) - y_t in ONE instruction, then a ScalarE
    activation(Square, accum_out=...) squares and row-sums in ONE instruction.
  - Tail chunks taper (8,4,2,1,1 blocks) and live in their own pool tags so
    the ring never stalls on slot releases at the end; trailing compute after
    the last DMA is ~1 row-block.
"""

import numpy as np

from concourse import bacc, bass, mybir
from concourse.bass_utils import run_bass_kernel_spmd
from concourse.tile import TileContext

N_CORES = 8
B = 131072
BINS = 256
ROWS = B // N_CORES  # 16384 rows per core
P = 128  # SBUF partitions
N_BLK = ROWS // P  # 128 row-blocks per core (one row per partition each)
# 8-row-block (2 MB) streaming chunks pipeline the DMA-completion semaphore
# latency best (measured vs 4/16/32-block variants); the tail tapers in
# dedicated pool slots so trailing compute after the last DMA is ~1 block.
HEAD = [8] * 14  # main-pool streaming chunks
CHUNK_SLOT = 8  # main io pool slot size in row-blocks
IO_BUFS = 8
TAIL = [8, 4, 2, 1, 1]  # dedicated slots each
CHUNKS = HEAD + TAIL
assert sum(CHUNKS) == N_BLK
C_BUFS = 16
SQ_BUFS = 8

F32 = mybir.dt.float32


def build_nc() -> bass.Bass:
    nc = bacc.Bacc()

    xy = nc.declare_dram_parameter("xy", [2, ROWS, BINS], F32, isOutput=False)
    out = nc.declare_dram_parameter("out", [ROWS], F32, isOutput=True)

    # [128, 2, N_BLK * BINS]; partition p holds rows p*N_BLK .. p*N_BLK+N_BLK-1
    xyv = xy[:].rearrange("z (p n) d -> p z (n d)", p=P)
    ov = out[:].rearrange("(p n) -> p n", p=P)  # [128, N_BLK]

    with (
        TileContext(nc) as tc,
        tc.tile_pool(name="io", bufs=IO_BUFS) as io_pool,
        tc.tile_pool(name="iotail", bufs=1) as tail_pool,
        tc.tile_pool(name="cdf", bufs=C_BUFS) as c_pool,
        tc.tile_pool(name="res", bufs=1) as res_pool,
        tc.tile_pool(name="sq", bufs=SQ_BUFS, space="PSUM") as sq_pool,
    ):
        out_sb = res_pool.tile([P, N_BLK], F32)

        # Warm the ACT Square table at t=0 so the ~2.7us table load overlaps
        # the first input DMAs instead of stalling the first real activation.
        warm = res_pool.tile([P, 1], F32, tag="warm")
        warm2 = res_pool.tile([P, 1], F32, tag="warm2")
        nc.vector.memset(warm[:], 0)
        nc.scalar.activation(
            out=warm2[:],
            in_=warm[:],
            func=mybir.ActivationFunctionType.Square,
        )

        blk0 = 0
        for ci, tsz in enumerate(CHUNKS):
            if ci < len(HEAD):
                slot = CHUNK_SLOT
                xyt = io_pool.tile(
                    [P, 2 * slot * BINS], F32, tag="xyt", name=f"xyt{ci}"
                )
            else:
                slot = tsz
                xyt = tail_pool.tile(
                    [P, 2 * slot * BINS], F32, tag=f"tail{ci}", name=f"xyt{ci}"
                )
            # [128, 2, tsz*256] view of the slot: x at free offset 0, y at
            # slot*BINS — matches the DRAM [p, z, f] chunk below.
            xyt3 = xyt[:].rearrange("p (z f) -> p z f", z=2)[:, :, : tsz * BINS]
            lo, hi = blk0 * BINS, (blk0 + tsz) * BINS
            nc.sync.dma_start(out=xyt3, in_=xyv[:, :, lo:hi])
            for t in range(tsz):
                col = blk0 + t
                xoff = t * BINS
                yoff = slot * BINS + t * BINS
                c = c_pool.tile([P, BINS], F32)
                nc.vector.tensor_tensor_scan(
                    out=c[:],
                    data0=xyt[:, xoff : xoff + BINS],
                    data1=xyt[:, yoff : yoff + BINS],
                    initial=0.0,
                    op0=mybir.AluOpType.add,
                    op1=mybir.AluOpType.subtract,
                )
                sq = sq_pool.tile([P, BINS], F32)
                nc.scalar.activation(
                    out=sq[:],
                    in_=c[:],
                    func=mybir.ActivationFunctionType.Square,
                    accum_out=out_sb[:, col : col + 1],
                )
            blk0 += tsz
        nc.sync.dma_start(out=ov[:, :], in_=out_sb[:])
    nc.finalize()
    return nc


_NC = None


def _get_nc() -> bass.Bass:
    global _NC
    if _NC is None:
        _NC = build_nc()
    return _NC


def kernel(x: np.ndarray, y: np.ndarray) -> np.ndarray:
    assert x.shape == (B, BINS) and y.shape == (B, BINS), (x.shape, y.shape)
    x = np.ascontiguousarray(x, dtype=np.float32)
    y = np.ascontiguousarray(y, dtype=np.float32)
    in_maps = []
    for i in range(N_CORES):
        sl = slice(i * ROWS, (i + 1) * ROWS)
        in_maps.append({"xy": np.stack([x[sl], y[sl]])})
    res = run_bass_kernel_spmd(_get_nc(), in_maps, list(range(N_CORES)))
    return np.concatenate([m["out"] for m in res.results])

